# revision 1
# baseline (speedup 1.0000x reference)
"""Trainium2 Bass kernel for nn_Bone_loss (VarLoss bone-length variance loss).

Strategy (pure data-parallel over 8 cores, 1024 samples each):
  - Each sample contributes 14 gathered scalars pred[s, jt] = output[s, ind[s,jt]].
    We use hardware-DGE indirect DMA (indirect_dma_start): the gpsimd
    sequencer expands one 4-byte descriptor per int32 offset read from an
    SBUF table (~1.1 ns/entry), and the Pool dynamic-queue rings execute
    them (~1.8 ns/entry across 16 SDMA engines).
  - HW behavior (measured): descriptor count = dest AP dim[-2]; descriptors
    land sequentially in ONE partition; the offset table is consumed
    COLUMN-major (down partitions, then next column). So slots are gathered
    linearly into partition 0 and the offset table is the TRANSPOSE of the
    sample-layout offsets (one PE identity-transpose, exact for ints < 2^24),
    then one 512B-per-partition spray restores [128, 128] sample layout.
  - Offsets = s*4096 + ind[s, jt] + BIG*(pad or target<=0.5), computed in f32
    (max < 2^24, exact). A bounds check skips BIG entries (invisible joints
    can't contribute to any bone), roughly halving executed descriptors.
  - The gather is split into 4 calls alternating between two Pool dynamic
    queues: expansions stream back-to-back on the sequencer while ring
    execution overlaps across queues. A large descriptor carveout keeps all
    calls in flight without the full-drain poll.
  - Sample layout: s = g*128 + p (partition p, group g in [0,8)); pred tile
    column u = g*16 + jtpos (2 pad slots per group). Bone math vectorized
    over [128 x (bone*8+g)] tiles; endpoint differences read pred directly
    via 7 piecewise-affine AP runs; sqrt sees w^2-scaled, visibility
    bit-masked d2 so skipped-slot garbage never propagates.
  - Per-core partial sum via PE ones-matmul -> host adds 8 partials, *0.5/B.
"""

import numpy as np

import concourse.bass as bass
import concourse.tile as tile
from concourse import bacc, mybir
from concourse.bass_utils import run_bass_kernel_spmd

NCORES = 8
B = 8192
S = B // NCORES          # samples per core
P = 128
G = 8                    # groups of 128 samples
UJ = 16                  # slots per group (16 = padded to power of two)
U = G * UJ               # within-partition slot count
N = P * U                # gather slots
_COLS = [32, 32, 32, 32]      # table-column split per gather call

_JL = [0, 1, 2, 3, 4, 5, 6, 8, 11, 12, 13, 14, 15, 16]      # joints used
# contiguous (jtpos0, joint0, len) runs of used joints
_JRUNS = [(0, 0, 7), (7, 8, 1), (8, 11, 6)]
# Bones reordered within groups so endpoint jtpos sequences form affine runs.
# Groups stay [0:4], [4:8], [8:10], [10:12].
_ID1 = [2, 3, 4, 5, 11, 12, 15, 16, 1, 4, 14, 11]
_ID2 = [1, 2, 5, 6, 12, 13, 14, 15, 0, 0, 8, 8]
_WB = [1.0, 1.0085885098415446, 1.0, 1.0085885098415446,
       1.0, 1.1375361376887123, 1.0, 1.1375361376887123,
       1.0, 1.0, 1.0, 1.0]
# affine runs (bone0, len, jtpos0, stride) per endpoint, for target/gt_2d
_RUNS_E1 = [(0, 4, 2, 1), (4, 2, 8, 1), (6, 2, 12, 1), (8, 1, 1, 1),
            (9, 1, 4, 1), (10, 1, 11, 1), (11, 1, 8, 1)]
_RUNS_E2 = [(0, 2, 1, 1), (2, 2, 5, 1), (4, 4, 9, 1), (8, 2, 0, 0),
            (10, 2, 7, 0)]
# joint endpoint-difference pieces (bone0, len, q1, st1, q2, st2):
# dp[b] = pred[q1+st1*k] - pred[q2+st2*k]
_DP_PIECES = [(0, 2, 2, 1, 1, 1), (2, 2, 4, 1, 5, 1), (4, 2, 8, 1, 9, 1),
              (6, 2, 12, 1, 11, 1), (8, 2, 1, 3, 0, 0),
              (10, 1, 11, 0, 7, 0), (11, 1, 8, 0, 7, 0)]
_VAR_WEIGHT = 1.0
_BIG = 8388608.0         # 2^23: skipped-offset marker, > bounds_check

_F32 = mybir.dt.float32
_I32 = mybir.dt.int32


def _ap(base_ap, dims, off=0):
    """Custom AP: keep base partition dim, override free dims; offset in elems."""
    return bass.AP(base_ap.tensor, base_ap.offset + off,
                   [list(base_ap.ap[0])] + [list(d) for d in dims])


def _dap(base_ap, dims, off=0):
    """Custom DRAM AP with ALL dims explicit (first dim included)."""
    return bass.AP(base_ap.tensor, base_ap.offset + off,
                   [list(d) for d in dims])


def _consts():
    p = np.arange(P, dtype=np.float32)
    g = np.arange(G, dtype=np.float32)
    c_pg = ((g[None, :] * 128 + p[:, None]) * 4096 + _BIG).astype(np.float32)
    c_id = np.eye(P, dtype=np.float32)
    c_w = np.broadcast_to(np.asarray(_WB, np.float32), (P, 12)).copy()
    c_one = np.ones((P, 1), np.float32)
    return {"c_pg": c_pg, "c_id": c_id, "c_w": c_w, "c_one": c_one}


def _build_nc():
    nc = bacc.Bacc("TRN2", target_bir_lowering=False, debug=False,
                   enable_asserts=False, num_devices=NCORES,
                   dynamic_dma_scratch_size=36864)
    outv = nc.dram_tensor("outv", [S * 4096, 1], _F32, kind="ExternalInput").ap()
    indv = nc.dram_tensor("indv", [S, 34], _I32, kind="ExternalInput").ap()
    tgtv = nc.dram_tensor("tgtv", [S, 17], _F32, kind="ExternalInput").ap()
    gxyv = nc.dram_tensor("gxyv", [S, 34], _F32, kind="ExternalInput").ap()
    mskv = nc.dram_tensor("mskv", [S, 17], _F32, kind="ExternalInput").ap()
    c_pg = nc.dram_tensor("c_pg", [P, G], _F32, kind="ExternalInput").ap()
    c_id = nc.dram_tensor("c_id", [P, P], _F32, kind="ExternalInput").ap()
    c_w = nc.dram_tensor("c_w", [P, 12], _F32, kind="ExternalInput").ap()
    c_one = nc.dram_tensor("c_one", [P, 1], _F32, kind="ExternalInput").ap()
    res = nc.dram_tensor("res", [1, 1], _F32, kind="ExternalOutput").ap()

    AL = mybir.AluOpType
    X = mybir.AxisListType.X
    with tile.TileContext(nc) as tc:
        with tc.tile_pool(name="sbuf", bufs=1) as pool, \
             tc.tile_pool(name="psum", bufs=1, space="PSUM") as psum_pool:
            # ---- loads (sync: gather-critical; scalar: the rest) ----
            pg_t = pool.tile([P, G], _F32)
            nc.sync.dma_start(pg_t[:], c_pg[:])
            ind_t = pool.tile([P, G * 34], _I32)
            nc.sync.dma_start(_ap(ind_t[:], [[34, G], [1, 34]]),
                              _dap(indv[:], [[34, P], [P * 34, G], [1, 34]]))
            tgt_t = pool.tile([P, G * 17], _F32)
            nc.sync.dma_start(_ap(tgt_t[:], [[17, G], [1, 17]]),
                              _dap(tgtv[:], [[17, P], [P * 17, G], [1, 17]]))
            id_t = pool.tile([P, P], _F32)
            nc.scalar.dma_start(id_t[:], c_id[:])
            gxy_t = pool.tile([P, G * 34], _F32)
            nc.scalar.dma_start(_ap(gxy_t[:], [[34, G], [1, 34]]),
                                _dap(gxyv[:], [[34, P], [P * 34, G], [1, 34]]))
            msk_t = pool.tile([P, G * 17], _F32)
            nc.scalar.dma_start(_ap(msk_t[:], [[17, G], [1, 17]]),
                                _dap(mskv[:], [[17, P], [P * 17, G], [1, 17]]))
            w_t = pool.tile([P, 12], _F32)
            nc.scalar.dma_start(w_t[:], c_w[:])
            one_t = pool.tile([P, 1], _F32)
            nc.scalar.dma_start(one_t[:], c_one[:])
            # prefetch the Sqrt activation table off the critical tail
            warm = pool.tile([1, 1], _F32)
            nc.vector.memset(warm[:], 1.0)
            nc.scalar.sqrt(out=warm[:], in_=warm[:])

            # ---- offsets in sample layout: offs[p, g*16+jt] ----
            # offs = ind + (s*4096 + BIG) + (tgt>0.5 ? -BIG : 0); pads stay BIG
            offs = pool.tile([P, U], _F32)
            nc.vector.memset(offs[:], _BIG)
            for (jp0, jo0, ln) in _JRUNS:
                nc.vector.tensor_copy(
                    out=_ap(offs[:], [[UJ, G], [1, ln]], off=jp0),
                    in_=_ap(ind_t[:], [[34, G], [2, ln]], off=2 * jo0))
            nc.vector.tensor_tensor(
                out=_ap(offs[:], [[UJ, G], [1, 14]]),
                in0=_ap(offs[:], [[UJ, G], [1, 14]]),
                in1=_ap(pg_t[:], [[1, G], [0, 14]]), op=AL.add)
            vjt = pool.tile([P, U], _F32)
            for (jp0, jo0, ln) in _JRUNS:
                nc.gpsimd.tensor_scalar(
                    out=_ap(vjt[:], [[UJ, G], [1, ln]], off=jp0),
                    in0=_ap(tgt_t[:], [[17, G], [1, ln]], off=jo0),
                    scalar1=0.5, scalar2=-_BIG, op0=AL.is_gt, op1=AL.mult)
            nc.vector.tensor_tensor(
                out=_ap(offs[:], [[UJ, G], [1, 14]]),
                in0=_ap(offs[:], [[UJ, G], [1, 14]]),
                in1=_ap(vjt[:], [[UJ, G], [1, 14]]), op=AL.add)

            # ---- transpose -> column-major offset table (int32) ----
            tp = psum_pool.tile([U, P], _F32, space="PSUM")
            nc.tensor.transpose(out=tp[:], in_=offs[:], identity=id_t[:])
            table = pool.tile([U, P], _I32)
            nc.vector.tensor_copy(out=table[:], in_=tp[:])

            # ---- indirect gather: 4-byte descriptors into partition 0 ----
            lins = []
            c0 = 0
            for k, cw in enumerate(_COLS):
                chunk = cw * U
                lin_k = pool.tile([P, chunk], _F32, name=f"lin{k}")
                lins.append((lin_k, c0, cw))
                gi = nc.gpsimd.indirect_dma_start(
                    out=_ap(lin_k[k:k + 1, :], [[1, chunk], [1, 1]]),
                    out_offset=None,
                    in_=outv[:],
                    in_offset=bass.IndirectOffsetOnAxis(
                        ap=table[:, c0:c0 + cw], axis=0),
                    bounds_check=S * 4096 - 1,
                    oob_is_err=False,
                )
                gi.ins.single_packet = True
                c0 += cw
            # ---- spray p0 linear -> pred[p, g*16+jt] ----
            predt = pool.tile([P, U], _F32)
            for k, (lin_k, c0, cw) in enumerate(lins):
                nc.sync.dma_start(
                    predt[c0:c0 + cw, :],
                    _ap(lin_k[k:k + 1, :], [[U, cw], [1, U]]))

            # ---- early work (overlaps gather): masks, gt_2d bone terms ----
            tc.tile_set_cur_wait(0.5)
            msum = pool.tile([P, G], _F32)
            nc.vector.tensor_reduce(out=msum[:],
                                    in_=_ap(msk_t[:], [[17, G], [1, 17]]),
                                    axis=X, op=AL.add)
            nc.vector.tensor_scalar(out=msum[:], in0=msum[:], scalar1=0.0,
                                    scalar2=None, op0=AL.is_equal)

            t1b = pool.tile([P, 96], _F32)
            t2b = pool.tile([P, 96], _F32)
            for e, (runs, dst) in enumerate(((_RUNS_E1, t1b), (_RUNS_E2, t2b))):
                for (b0, ln, q0, st) in runs:
                    nc.vector.tensor_copy(
                        out=_ap(dst[:], [[8, ln], [1, 8]], off=b0 * 8),
                        in_=_ap(tgt_t[:], [[st, ln], [17, 8]], off=_JL[q0]))
            v1 = pool.tile([P, 96], _F32)
            v2 = pool.tile([P, 96], _F32)
            vis = pool.tile([P, 96], _F32)
            nc.vector.tensor_scalar(out=v1[:], in0=t1b[:], scalar1=0.5,
                                    scalar2=None, op0=AL.is_gt)
            nc.vector.tensor_scalar(out=v2[:], in0=t2b[:], scalar1=0.5,
                                    scalar2=None, op0=AL.is_gt)
            nc.vector.tensor_tensor(out=vis[:], in0=v1[:], in1=v2[:], op=AL.mult)
            # visibility as an all-ones/zeros int mask for d2 sanitization
            vmi = pool.tile([P, 96], _I32)
            nc.vector.tensor_copy(out=vmi[:], in_=vis[:])
            nc.vector.tensor_scalar(out=vmi[:], in0=vmi[:], scalar1=-1,
                                    scalar2=None, op0=AL.mult)

            gxyb = pool.tile([P, 384], _F32)   # [e*192 + b*16 + xy*8 + g]
            for e, runs in enumerate((_RUNS_E1, _RUNS_E2)):
                for (b0, ln, q0, st) in runs:
                    nc.vector.tensor_copy(
                        out=_ap(gxyb[:], [[16, ln], [8, 2], [1, 8]],
                                off=e * 192 + b0 * 16),
                        in_=_ap(gxy_t[:], [[2 * st, ln], [1, 2], [34, 8]],
                                off=2 * _JL[q0]))
            dx = pool.tile([P, 96], _F32)
            dy = pool.tile([P, 96], _F32)
            xy2 = pool.tile([P, 96], _F32)
            nc.vector.tensor_tensor(
                out=dx[:].rearrange("p (a b) -> p a b", a=12),
                in0=_ap(gxyb[:], [[16, 12], [1, 8]], off=0),
                in1=_ap(gxyb[:], [[16, 12], [1, 8]], off=192), op=AL.subtract)
            nc.vector.tensor_tensor(
                out=dy[:].rearrange("p (a b) -> p a b", a=12),
                in0=_ap(gxyb[:], [[16, 12], [1, 8]], off=8),
                in1=_ap(gxyb[:], [[16, 12], [1, 8]], off=200), op=AL.subtract)
            nc.vector.tensor_tensor(out=dx[:], in0=dx[:], in1=dx[:], op=AL.mult)
            nc.vector.tensor_tensor(out=dy[:], in0=dy[:], in1=dy[:], op=AL.mult)
            nc.vector.tensor_tensor(out=xy2[:], in0=dx[:], in1=dy[:], op=AL.add)
            # fold w into xy2 (ell = sqrt((w*dp)^2 + w^2*xy2))
            w2 = pool.tile([P, 96], _F32)
            nc.vector.tensor_tensor(
                out=w2[:].rearrange("p (a b) -> p a b", a=12),
                in0=_ap(w_t[:], [[1, 12], [0, 8]]),
                in1=_ap(w_t[:], [[1, 12], [0, 8]]), op=AL.mult)
            nc.vector.tensor_tensor(out=xy2[:], in0=xy2[:], in1=w2[:],
                                    op=AL.mult)

            # ---- late bone math (needs pred) ----
            tc.tile_set_cur_wait(1.0)
            dp = pool.tile([P, 96], _F32)
            for (b0, ln, q1, st1, q2, st2) in _DP_PIECES:
                nc.vector.tensor_tensor(
                    out=_ap(dp[:], [[8, ln], [1, 8]], off=b0 * 8),
                    in0=_ap(predt[:], [[st1, ln], [UJ, 8]], off=q1),
                    in1=_ap(predt[:], [[st2, ln], [UJ, 8]], off=q2),
                    op=AL.subtract)
            nc.vector.tensor_tensor(
                out=dp[:].rearrange("p (a b) -> p a b", a=12),
                in0=dp[:].rearrange("p (a b) -> p a b", a=12),
                in1=_ap(w_t[:], [[1, 12], [0, 8]]), op=AL.mult)
            nc.vector.tensor_tensor(out=dp[:], in0=dp[:], in1=dp[:], op=AL.mult)
            nc.vector.tensor_tensor(out=dp[:], in0=dp[:], in1=xy2[:], op=AL.add)
            # bit-mask d2 with bone visibility so sqrt never sees garbage
            nc.vector.tensor_tensor(out=dp[:].bitcast(_I32),
                                    in0=dp[:].bitcast(_I32), in1=vmi[:],
                                    op=AL.bitwise_and)
            ell = pool.tile([P, 96], _F32)
            nc.scalar.sqrt(out=ell[:], in_=dp[:])
            # per-group mean E = sum_l / max(num,1) via reciprocal
            sum_l = pool.tile([P, 32], _F32)
            num = pool.tile([P, 32], _F32)
            for (src_t, dst_t) in ((ell, sum_l), (vis, num)):
                nc.vector.tensor_reduce(
                    out=_ap(dst_t[:], [[8, 2], [1, 8]]),
                    in_=_ap(src_t[:], [[32, 2], [1, 8], [8, 4]]),
                    axis=X, op=AL.add)
                nc.vector.tensor_reduce(
                    out=_ap(dst_t[:], [[8, 2], [1, 8]], off=16),
                    in_=_ap(src_t[:], [[16, 2], [1, 8], [8, 2]], off=64),
                    axis=X, op=AL.add)
            nc.vector.tensor_scalar(out=num[:], in0=num[:], scalar1=1.0,
                                    scalar2=None, op0=AL.max)
            rn = pool.tile([P, 32], _F32)
            nc.vector.reciprocal(out=rn[:], in_=num[:])
            e_t = pool.tile([P, 32], _F32)
            nc.vector.tensor_tensor(out=e_t[:], in0=sum_l[:], in1=rn[:],
                                    op=AL.mult)
            eb = pool.tile([P, 96], _F32)
            nb = pool.tile([P, 96], _F32)
            for (src_t, dst_t) in ((e_t, eb), (rn, nb)):
                nc.vector.tensor_copy(
                    out=_ap(dst_t[:], [[32, 2], [8, 4], [1, 8]]),
                    in_=_ap(src_t[:], [[8, 2], [0, 4], [1, 8]]))
                nc.vector.tensor_copy(
                    out=_ap(dst_t[:], [[16, 2], [8, 2], [1, 8]], off=64),
                    in_=_ap(src_t[:], [[8, 2], [0, 2], [1, 8]], off=16))
            # contrib = (ell>0) * (ell-E)^2 / num; global *0.5 on host
            gt = pool.tile([P, 96], _F32)
            nc.vector.tensor_scalar(out=gt[:], in0=ell[:], scalar1=0.0,
                                    scalar2=None, op0=AL.is_gt)
            nc.vector.tensor_tensor(out=eb[:], in0=ell[:], in1=eb[:],
                                    op=AL.subtract)
            nc.vector.tensor_tensor(out=eb[:], in0=eb[:], in1=eb[:], op=AL.mult)
            nc.vector.tensor_tensor(out=eb[:], in0=eb[:], in1=nb[:], op=AL.mult)
            nc.vector.tensor_tensor(out=eb[:], in0=eb[:], in1=gt[:], op=AL.mult)
            # per-lane sums, active mask, cross-partition total via PE
            pl = pool.tile([P, G], _F32)
            nc.vector.tensor_reduce(out=pl[:],
                                    in_=_ap(eb[:], [[1, 8], [8, 12]]),
                                    axis=X, op=AL.add)
            nc.vector.tensor_tensor(out=pl[:], in0=pl[:], in1=msum[:],
                                    op=AL.mult)
            ps = psum_pool.tile([1, G], _F32, space="PSUM")
            nc.tensor.matmul(out=ps[:], lhsT=one_t[:], rhs=pl[:],
                             start=True, stop=True)
            tot = pool.tile([1, 1], _F32)
            nc.vector.tensor_reduce(out=tot[:], in_=ps[:], axis=X, op=AL.add)
            nc.sync.dma_start(res[:], tot[0:1, :])
    nc.compile()
    return nc


_NC_CACHE = None
LAST_RESULTS = None


def kernel(output, mask, ind, target, gt_2d):
    global _NC_CACHE, LAST_RESULTS
    if _NC_CACHE is None:
        _NC_CACHE = _build_nc()
    nc = _NC_CACHE

    output = np.ascontiguousarray(np.asarray(output), dtype=np.float32)
    mask = np.ascontiguousarray(np.asarray(mask), dtype=np.float32)
    target = np.ascontiguousarray(np.asarray(target), dtype=np.float32)
    gt_2d = np.ascontiguousarray(np.asarray(gt_2d), dtype=np.float32)
    ind = np.ascontiguousarray(np.asarray(ind))
    if ind.dtype != np.int64:
        ind = ind.astype(np.int64)

    consts = _consts()
    in_maps = []
    for c in range(NCORES):
        sl = slice(c * S, (c + 1) * S)
        in_maps.append({
            "outv": np.ascontiguousarray(output[sl]).reshape(S * 4096, 1),
            "indv": np.ascontiguousarray(ind[sl]).view(np.int32).reshape(S, 34),
            "tgtv": np.ascontiguousarray(target[sl]),
            "gxyv": np.ascontiguousarray(gt_2d[sl]).reshape(S, 34),
            "mskv": np.ascontiguousarray(mask[sl]),
            **consts,
        })
    res = run_bass_kernel_spmd(nc, in_maps, core_ids=list(range(NCORES)))
    LAST_RESULTS = res
    total = sum(float(res.results[c]["res"][0, 0]) for c in range(NCORES))
    return np.asarray([_VAR_WEIGHT * total * 0.5 / B], dtype=np.float32)



# revision 3
# speedup vs baseline: 1.0485x; 1.0485x over previous
"""Trainium2 Bass kernel for nn_Bone_loss (VarLoss bone-length variance loss).

Strategy (pure data-parallel over 8 cores, 1024 samples each):
  - Each sample contributes 14 gathered scalars pred[s, jt] = output[s, ind[s,jt]].
    We use hardware-DGE indirect DMA (indirect_dma_start): the gpsimd
    sequencer expands one 4-byte descriptor per int32 offset read from an
    SBUF table (~1.1 ns/entry), and the Pool dynamic-queue rings execute
    them (~1.8 ns/entry across 16 SDMA engines).
  - HW behavior (measured): descriptor count = dest AP dim[-2]; descriptors
    land sequentially in ONE partition; the offset table is consumed
    COLUMN-major (down partitions, then next column). So slots are gathered
    linearly into partition 0 and the offset table is the TRANSPOSE of the
    sample-layout offsets (one PE identity-transpose, exact for ints < 2^24),
    then one 512B-per-partition spray restores [128, 128] sample layout.
  - Offsets = s*4096 + ind[s, jt] + BIG*(pad or target<=0.5), computed in f32
    (max < 2^24, exact). A bounds check skips BIG entries (invisible joints
    can't contribute to any bone), roughly halving executed descriptors.
  - The gather is split into 4 calls alternating between two Pool dynamic
    queues: expansions stream back-to-back on the sequencer while ring
    execution overlaps across queues. A large descriptor carveout keeps all
    calls in flight without the full-drain poll.
  - Sample layout: s = g*128 + p (partition p, group g in [0,8)); pred tile
    column u = g*16 + jtpos (2 pad slots per group). Bone math vectorized
    over [128 x (bone*8+g)] tiles; endpoint differences read pred directly
    via 7 piecewise-affine AP runs; sqrt sees w^2-scaled, visibility
    bit-masked d2 so skipped-slot garbage never propagates.
  - Per-core partial sum via PE ones-matmul -> host adds 8 partials, *0.5/B.
"""

import numpy as np

import concourse.bass as bass
import concourse.tile as tile
from concourse import bacc, mybir
from concourse.bass_utils import run_bass_kernel_spmd

NCORES = 8
B = 8192
S = B // NCORES          # samples per core
P = 128
G = 8                    # groups of 128 samples
UJ = 16                  # slots per group (16 = padded to power of two)
U = G * UJ               # within-partition slot count
N = P * U                # gather slots
_COLS = [32, 32, 32, 32]      # table-column split per gather call

_JL = [0, 1, 2, 3, 4, 5, 6, 8, 11, 12, 13, 14, 15, 16]      # joints used
# contiguous (jtpos0, joint0, len) runs of used joints
_JRUNS = [(0, 0, 7), (7, 8, 1), (8, 11, 6)]
# Bones reordered within groups so endpoint jtpos sequences form affine runs.
# Groups stay [0:4], [4:8], [8:10], [10:12].
_ID1 = [2, 3, 4, 5, 11, 12, 15, 16, 1, 4, 14, 11]
_ID2 = [1, 2, 5, 6, 12, 13, 14, 15, 0, 0, 8, 8]
_WB = [1.0, 1.0085885098415446, 1.0, 1.0085885098415446,
       1.0, 1.1375361376887123, 1.0, 1.1375361376887123,
       1.0, 1.0, 1.0, 1.0]
# affine runs (bone0, len, jtpos0, stride) per endpoint, for target/gt_2d
_RUNS_E1 = [(0, 4, 2, 1), (4, 2, 8, 1), (6, 2, 12, 1), (8, 1, 1, 1),
            (9, 1, 4, 1), (10, 1, 11, 1), (11, 1, 8, 1)]
_RUNS_E2 = [(0, 2, 1, 1), (2, 2, 5, 1), (4, 4, 9, 1), (8, 2, 0, 0),
            (10, 2, 7, 0)]
# joint endpoint-difference pieces (bone0, len, q1, st1, q2, st2):
# dp[b] = pred[q1+st1*k] - pred[q2+st2*k]
_DP_PIECES = [(0, 2, 2, 1, 1, 1), (2, 2, 4, 1, 5, 1), (4, 2, 8, 1, 9, 1),
              (6, 2, 12, 1, 11, 1), (8, 2, 1, 3, 0, 0),
              (10, 1, 11, 0, 7, 0), (11, 1, 8, 0, 7, 0)]
_VAR_WEIGHT = 1.0
_BIG = 8388608.0         # 2^23: skipped-offset marker, > bounds_check

_F32 = mybir.dt.float32
_I32 = mybir.dt.int32


def _ap(base_ap, dims, off=0):
    """Custom AP: keep base partition dim, override free dims; offset in elems."""
    return bass.AP(base_ap.tensor, base_ap.offset + off,
                   [list(base_ap.ap[0])] + [list(d) for d in dims])


def _dap(base_ap, dims, off=0):
    """Custom DRAM AP with ALL dims explicit (first dim included)."""
    return bass.AP(base_ap.tensor, base_ap.offset + off,
                   [list(d) for d in dims])


def _consts():
    p = np.arange(P, dtype=np.float32)
    g = np.arange(G, dtype=np.float32)
    c_pg = ((g[None, :] * 128 + p[:, None]) * 4096 + _BIG).astype(np.float32)
    c_id = np.eye(P, dtype=np.float32)
    c_w = np.broadcast_to(np.asarray(_WB, np.float32), (P, 12)).copy()
    c_one = np.ones((P, 1), np.float32)
    return {"c_pg": c_pg, "c_id": c_id, "c_w": c_w, "c_one": c_one}


def _build_nc():
    nc = bacc.Bacc("TRN2", target_bir_lowering=False, debug=False,
                   enable_asserts=False, num_devices=NCORES,
                   dynamic_dma_scratch_size=36864, num_swdge_queues=4)
    outv = nc.dram_tensor("outv", [S * 4096, 1], _F32, kind="ExternalInput").ap()
    indv = nc.dram_tensor("indv", [S, 34], _I32, kind="ExternalInput").ap()
    tgtv = nc.dram_tensor("tgtv", [S, 17], _F32, kind="ExternalInput").ap()
    gxyv = nc.dram_tensor("gxyv", [S, 34], _F32, kind="ExternalInput").ap()
    mskv = nc.dram_tensor("mskv", [S, 17], _F32, kind="ExternalInput").ap()
    c_pg = nc.dram_tensor("c_pg", [P, G], _F32, kind="ExternalInput").ap()
    c_id = nc.dram_tensor("c_id", [P, P], _F32, kind="ExternalInput").ap()
    c_w = nc.dram_tensor("c_w", [P, 12], _F32, kind="ExternalInput").ap()
    c_one = nc.dram_tensor("c_one", [P, 1], _F32, kind="ExternalInput").ap()
    res = nc.dram_tensor("res", [1, 1], _F32, kind="ExternalOutput").ap()

    AL = mybir.AluOpType
    X = mybir.AxisListType.X
    with tile.TileContext(nc) as tc:
        with tc.tile_pool(name="sbuf", bufs=1) as pool, \
             tc.tile_pool(name="psum", bufs=1, space="PSUM") as psum_pool:
            # ---- loads (sync: gather-critical; scalar: the rest) ----
            pg_t = pool.tile([P, G], _F32)
            nc.sync.dma_start(pg_t[:], c_pg[:])
            ind_t = pool.tile([P, G * 34], _I32)
            nc.sync.dma_start(_ap(ind_t[:], [[34, G], [1, 34]]),
                              _dap(indv[:], [[34, P], [P * 34, G], [1, 34]]))
            tgt_t = pool.tile([P, G * 17], _F32)
            nc.sync.dma_start(_ap(tgt_t[:], [[17, G], [1, 17]]),
                              _dap(tgtv[:], [[17, P], [P * 17, G], [1, 17]]))
            id_t = pool.tile([P, P], _F32)
            nc.scalar.dma_start(id_t[:], c_id[:])
            gxy_t = pool.tile([P, G * 34], _F32)
            nc.scalar.dma_start(_ap(gxy_t[:], [[34, G], [1, 34]]),
                                _dap(gxyv[:], [[34, P], [P * 34, G], [1, 34]]))
            msk_t = pool.tile([P, G * 17], _F32)
            nc.scalar.dma_start(_ap(msk_t[:], [[17, G], [1, 17]]),
                                _dap(mskv[:], [[17, P], [P * 17, G], [1, 17]]))
            w_t = pool.tile([P, 12], _F32)
            nc.scalar.dma_start(w_t[:], c_w[:])
            one_t = pool.tile([P, 1], _F32)
            nc.scalar.dma_start(one_t[:], c_one[:])
            # prefetch the Sqrt activation table off the critical tail
            warm = pool.tile([1, 1], _F32)
            nc.vector.memset(warm[:], 1.0)
            nc.scalar.sqrt(out=warm[:], in_=warm[:])

            # ---- offsets in sample layout: offs[p, g*16+jt] ----
            # offs = ind + (s*4096 + BIG) + (tgt>0.5 ? -BIG : 0); pads stay BIG
            offs = pool.tile([P, U], _F32)
            nc.vector.memset(offs[:], _BIG)
            for (jp0, jo0, ln) in _JRUNS:
                nc.vector.tensor_copy(
                    out=_ap(offs[:], [[UJ, G], [1, ln]], off=jp0),
                    in_=_ap(ind_t[:], [[34, G], [2, ln]], off=2 * jo0))
            nc.vector.tensor_tensor(
                out=_ap(offs[:], [[UJ, G], [1, 14]]),
                in0=_ap(offs[:], [[UJ, G], [1, 14]]),
                in1=_ap(pg_t[:], [[1, G], [0, 14]]), op=AL.add)
            vjt = pool.tile([P, U], _F32)
            for (jp0, jo0, ln) in _JRUNS:
                nc.gpsimd.tensor_scalar(
                    out=_ap(vjt[:], [[UJ, G], [1, ln]], off=jp0),
                    in0=_ap(tgt_t[:], [[17, G], [1, ln]], off=jo0),
                    scalar1=0.5, scalar2=-_BIG, op0=AL.is_gt, op1=AL.mult)
            nc.vector.tensor_tensor(
                out=_ap(offs[:], [[UJ, G], [1, 14]]),
                in0=_ap(offs[:], [[UJ, G], [1, 14]]),
                in1=_ap(vjt[:], [[UJ, G], [1, 14]]), op=AL.add)

            # ---- transpose -> column-major offset table (int32) ----
            tp = psum_pool.tile([U, P], _F32, space="PSUM")
            nc.tensor.transpose(out=tp[:], in_=offs[:], identity=id_t[:])
            table = pool.tile([U, P], _I32)
            nc.vector.tensor_copy(out=table[:], in_=tp[:])

            # ---- indirect gather: 4-byte descriptors into partition 0 ----
            lins = []
            c0 = 0
            for k, cw in enumerate(_COLS):
                chunk = cw * U
                lin_k = pool.tile([P, chunk], _F32, name=f"lin{k}")
                lins.append((lin_k, c0, cw))
                gi = nc.gpsimd.indirect_dma_start(
                    out=_ap(lin_k[k:k + 1, :], [[1, chunk], [1, 1]]),
                    out_offset=None,
                    in_=outv[:],
                    in_offset=bass.IndirectOffsetOnAxis(
                        ap=table[:, c0:c0 + cw], axis=0),
                    bounds_check=S * 4096 - 1,
                    oob_is_err=False,
                )
                gi.ins.single_packet = True
                gi.ins.queue = f"qPoolDynamic{k or ''}"
                c0 += cw
            # ---- spray p0 linear -> pred[p, g*16+jt] ----
            predt = pool.tile([P, U], _F32)
            for k, (lin_k, c0, cw) in enumerate(lins):
                nc.sync.dma_start(
                    predt[c0:c0 + cw, :],
                    _ap(lin_k[k:k + 1, :], [[U, cw], [1, U]]))

            # ---- early work (overlaps gather): masks, gt_2d bone terms ----
            tc.tile_set_cur_wait(0.5)
            msum = pool.tile([P, G], _F32)
            nc.vector.tensor_reduce(out=msum[:],
                                    in_=_ap(msk_t[:], [[17, G], [1, 17]]),
                                    axis=X, op=AL.add)
            nc.vector.tensor_scalar(out=msum[:], in0=msum[:], scalar1=0.0,
                                    scalar2=None, op0=AL.is_equal)

            t1b = pool.tile([P, 96], _F32)
            t2b = pool.tile([P, 96], _F32)
            for e, (runs, dst) in enumerate(((_RUNS_E1, t1b), (_RUNS_E2, t2b))):
                for (b0, ln, q0, st) in runs:
                    nc.vector.tensor_copy(
                        out=_ap(dst[:], [[8, ln], [1, 8]], off=b0 * 8),
                        in_=_ap(tgt_t[:], [[st, ln], [17, 8]], off=_JL[q0]))
            v1 = pool.tile([P, 96], _F32)
            v2 = pool.tile([P, 96], _F32)
            vis = pool.tile([P, 96], _F32)
            nc.vector.tensor_scalar(out=v1[:], in0=t1b[:], scalar1=0.5,
                                    scalar2=None, op0=AL.is_gt)
            nc.vector.tensor_scalar(out=v2[:], in0=t2b[:], scalar1=0.5,
                                    scalar2=None, op0=AL.is_gt)
            nc.vector.tensor_tensor(out=vis[:], in0=v1[:], in1=v2[:], op=AL.mult)
            # visibility as an all-ones/zeros int mask for d2 sanitization
            vmi = pool.tile([P, 96], _I32)
            nc.vector.tensor_copy(out=vmi[:], in_=vis[:])
            nc.vector.tensor_scalar(out=vmi[:], in0=vmi[:], scalar1=-1,
                                    scalar2=None, op0=AL.mult)

            gxyb = pool.tile([P, 384], _F32)   # [e*192 + b*16 + xy*8 + g]
            for e, runs in enumerate((_RUNS_E1, _RUNS_E2)):
                for (b0, ln, q0, st) in runs:
                    nc.vector.tensor_copy(
                        out=_ap(gxyb[:], [[16, ln], [8, 2], [1, 8]],
                                off=e * 192 + b0 * 16),
                        in_=_ap(gxy_t[:], [[2 * st, ln], [1, 2], [34, 8]],
                                off=2 * _JL[q0]))
            dx = pool.tile([P, 96], _F32)
            dy = pool.tile([P, 96], _F32)
            xy2 = pool.tile([P, 96], _F32)
            nc.vector.tensor_tensor(
                out=dx[:].rearrange("p (a b) -> p a b", a=12),
                in0=_ap(gxyb[:], [[16, 12], [1, 8]], off=0),
                in1=_ap(gxyb[:], [[16, 12], [1, 8]], off=192), op=AL.subtract)
            nc.vector.tensor_tensor(
                out=dy[:].rearrange("p (a b) -> p a b", a=12),
                in0=_ap(gxyb[:], [[16, 12], [1, 8]], off=8),
                in1=_ap(gxyb[:], [[16, 12], [1, 8]], off=200), op=AL.subtract)
            nc.vector.tensor_tensor(out=dx[:], in0=dx[:], in1=dx[:], op=AL.mult)
            nc.vector.tensor_tensor(out=dy[:], in0=dy[:], in1=dy[:], op=AL.mult)
            nc.vector.tensor_tensor(out=xy2[:], in0=dx[:], in1=dy[:], op=AL.add)
            # fold w into xy2 (ell = sqrt((w*dp)^2 + w^2*xy2))
            w2 = pool.tile([P, 96], _F32)
            nc.vector.tensor_tensor(
                out=w2[:].rearrange("p (a b) -> p a b", a=12),
                in0=_ap(w_t[:], [[1, 12], [0, 8]]),
                in1=_ap(w_t[:], [[1, 12], [0, 8]]), op=AL.mult)
            nc.vector.tensor_tensor(out=xy2[:], in0=xy2[:], in1=w2[:],
                                    op=AL.mult)

            # ---- late bone math (needs pred) ----
            tc.tile_set_cur_wait(1.0)
            dp = pool.tile([P, 96], _F32)
            for (b0, ln, q1, st1, q2, st2) in _DP_PIECES:
                nc.vector.tensor_tensor(
                    out=_ap(dp[:], [[8, ln], [1, 8]], off=b0 * 8),
                    in0=_ap(predt[:], [[st1, ln], [UJ, 8]], off=q1),
                    in1=_ap(predt[:], [[st2, ln], [UJ, 8]], off=q2),
                    op=AL.subtract)
            nc.vector.tensor_tensor(
                out=dp[:].rearrange("p (a b) -> p a b", a=12),
                in0=dp[:].rearrange("p (a b) -> p a b", a=12),
                in1=_ap(w_t[:], [[1, 12], [0, 8]]), op=AL.mult)
            nc.vector.tensor_tensor(out=dp[:], in0=dp[:], in1=dp[:], op=AL.mult)
            nc.vector.tensor_tensor(out=dp[:], in0=dp[:], in1=xy2[:], op=AL.add)
            # bit-mask d2 with bone visibility so sqrt never sees garbage
            nc.vector.tensor_tensor(out=dp[:].bitcast(_I32),
                                    in0=dp[:].bitcast(_I32), in1=vmi[:],
                                    op=AL.bitwise_and)
            ell = pool.tile([P, 96], _F32)
            nc.scalar.sqrt(out=ell[:], in_=dp[:])
            # per-group mean E = sum_l / max(num,1) via reciprocal
            sum_l = pool.tile([P, 32], _F32)
            num = pool.tile([P, 32], _F32)
            for (src_t, dst_t) in ((ell, sum_l), (vis, num)):
                nc.vector.tensor_reduce(
                    out=_ap(dst_t[:], [[8, 2], [1, 8]]),
                    in_=_ap(src_t[:], [[32, 2], [1, 8], [8, 4]]),
                    axis=X, op=AL.add)
                nc.vector.tensor_reduce(
                    out=_ap(dst_t[:], [[8, 2], [1, 8]], off=16),
                    in_=_ap(src_t[:], [[16, 2], [1, 8], [8, 2]], off=64),
                    axis=X, op=AL.add)
            nc.vector.tensor_scalar(out=num[:], in0=num[:], scalar1=1.0,
                                    scalar2=None, op0=AL.max)
            rn = pool.tile([P, 32], _F32)
            nc.vector.reciprocal(out=rn[:], in_=num[:])
            e_t = pool.tile([P, 32], _F32)
            nc.vector.tensor_tensor(out=e_t[:], in0=sum_l[:], in1=rn[:],
                                    op=AL.mult)
            eb = pool.tile([P, 96], _F32)
            nb = pool.tile([P, 96], _F32)
            for (src_t, dst_t) in ((e_t, eb), (rn, nb)):
                nc.vector.tensor_copy(
                    out=_ap(dst_t[:], [[32, 2], [8, 4], [1, 8]]),
                    in_=_ap(src_t[:], [[8, 2], [0, 4], [1, 8]]))
                nc.vector.tensor_copy(
                    out=_ap(dst_t[:], [[16, 2], [8, 2], [1, 8]], off=64),
                    in_=_ap(src_t[:], [[8, 2], [0, 2], [1, 8]], off=16))
            # contrib = (ell>0) * (ell-E)^2 / num; global *0.5 on host
            gt = pool.tile([P, 96], _F32)
            nc.vector.tensor_scalar(out=gt[:], in0=ell[:], scalar1=0.0,
                                    scalar2=None, op0=AL.is_gt)
            nc.vector.tensor_tensor(out=eb[:], in0=ell[:], in1=eb[:],
                                    op=AL.subtract)
            nc.vector.tensor_tensor(out=eb[:], in0=eb[:], in1=eb[:], op=AL.mult)
            nc.vector.tensor_tensor(out=eb[:], in0=eb[:], in1=nb[:], op=AL.mult)
            nc.vector.tensor_tensor(out=eb[:], in0=eb[:], in1=gt[:], op=AL.mult)
            # per-lane sums, active mask, cross-partition total via PE
            pl = pool.tile([P, G], _F32)
            nc.vector.tensor_reduce(out=pl[:],
                                    in_=_ap(eb[:], [[1, 8], [8, 12]]),
                                    axis=X, op=AL.add)
            nc.vector.tensor_tensor(out=pl[:], in0=pl[:], in1=msum[:],
                                    op=AL.mult)
            ps = psum_pool.tile([1, G], _F32, space="PSUM")
            nc.tensor.matmul(out=ps[:], lhsT=one_t[:], rhs=pl[:],
                             start=True, stop=True)
            tot = pool.tile([1, 1], _F32)
            nc.vector.tensor_reduce(out=tot[:], in_=ps[:], axis=X, op=AL.add)
            nc.sync.dma_start(res[:], tot[0:1, :])
    nc.compile()
    return nc


_NC_CACHE = None
LAST_RESULTS = None


def kernel(output, mask, ind, target, gt_2d):
    global _NC_CACHE, LAST_RESULTS
    if _NC_CACHE is None:
        _NC_CACHE = _build_nc()
    nc = _NC_CACHE

    output = np.ascontiguousarray(np.asarray(output), dtype=np.float32)
    mask = np.ascontiguousarray(np.asarray(mask), dtype=np.float32)
    target = np.ascontiguousarray(np.asarray(target), dtype=np.float32)
    gt_2d = np.ascontiguousarray(np.asarray(gt_2d), dtype=np.float32)
    ind = np.ascontiguousarray(np.asarray(ind))
    if ind.dtype != np.int64:
        ind = ind.astype(np.int64)

    consts = _consts()
    in_maps = []
    for c in range(NCORES):
        sl = slice(c * S, (c + 1) * S)
        in_maps.append({
            "outv": np.ascontiguousarray(output[sl]).reshape(S * 4096, 1),
            "indv": np.ascontiguousarray(ind[sl]).view(np.int32).reshape(S, 34),
            "tgtv": np.ascontiguousarray(target[sl]),
            "gxyv": np.ascontiguousarray(gt_2d[sl]).reshape(S, 34),
            "mskv": np.ascontiguousarray(mask[sl]),
            **consts,
        })
    res = run_bass_kernel_spmd(nc, in_maps, core_ids=list(range(NCORES)))
    LAST_RESULTS = res
    total = sum(float(res.results[c]["res"][0, 0]) for c in range(NCORES))
    return np.asarray([_VAR_WEIGHT * total * 0.5 / B], dtype=np.float32)

